# revision 10
# baseline (speedup 1.0000x reference)
"""
MinibatchDiscrimination kernel for 8x TRN2 NeuronCores (Bass/Tile).

Math:  x = inputs @ T  -> [B, K, D] with B=512, K=100, D=5
       out[i,k] = sum_j exp(-sum_d |x[i,k,d]-x[j,k,d]|)

Strategy — symmetric block-tournament over the pairwise matrix:

  The B x B pairwise matrix is tiled into 8x8 blocks of 64x64 (one row-group
  per core). Each unordered block-pair only needs computing once: from one
  computed block, ROW sums come from the ACT accumulator and COLUMN sums
  (= row sums of the transposed block, by symmetry of the L1 distance) come
  from a PE identity-matmul accumulation over the exp tiles. Core c computes
  blocks (c, c+k) for k=0..4 (mod 8, W=320 columns of its rolled copy):

    - diag block (k=0): row sums only (colsum would double-count by symmetry)
    - k=1,2,3: row sums kept locally + column sums exchanged to core c+k
      (exchange happens on the host during output assembly)
    - k=4: row sums only; the mirror pair {c, c+4} is computed independently
      by core c+4 as ITS k=4 block (distance-4 blocks are duplicated so the
      SPMD program stays identical across cores)

  Row j of core q then receives: own row sums (col-groups q..q+4) plus
  exchanged column sums from cores q-1, q-2, q-3 — all 8 groups exactly once.

Per core c of 8 (rolled by 64c so the program is SPMD-identical):
  - xT[kd, i] = sum_f T[f, kd] * inT[f, i] on PE (4 chunks of 125 kd), i<320.
  - Per output row j in 0..63:
      ab_c[p, i] = |xT_c[p, i] - xT_c[p, j]|   (DVE tensor_scalar
                   (subtract, abs_max vs 0.0) — fp16, 4x perf mode;
                   the per-partition scalar is an f32 upcast of the fp16 xT
                   column so the diagonal is exactly 0)
      dist[32c+m, :] = sum_d ab[5m+d, :]       (PE d-sum matmul with a
                   0/1 block matrix, col-tiled per chunk — no S-term or
                   negI matmul needed since abs values sum directly)
      dump[:, :]  = exp(-dist), fp16 -> SBUF   (ACT, accum_out gives the
                   row sums over all 320 cols in one pass)
      colacc     += dump[:, 64:256]            (PE identity matmul
                   accumulating in PSUM across all 64 j — the k=1,2,3
                   column sums, emitted 2 iterations late to pipeline)
  - dist row p=32c+m holds k=25c+m (m<25); host transposes/reassembles and
    adds the exchanged column-sum blocks.

  Hardware notes (CoreSim cost model, validated on TRN2 previously):
  - Steady state is ACT-bound: exp main pass 0.833*320+185 = 452ns plus the
    fixed 287ns accumulator-read = ~739ns/row; DVE 4x tensor_scalars at
    143.8ns = 575ns/row and PE 4 d-sums + colacc = 613ns/row overlap under
    it. 64 rows -> ~47us steady.
  - ab/dump tiles are STATIC rings sized to the whole loop (256 ab tiles,
    ~160KB of SBUF) so there are no cross-iteration WAW deps at all: DVE
    instructions carry no waits in steady state (the baseline lost ~10us+
    to 242 same-engine WAW EventSemaphores from rotating small rings).
  - Inputs land in 4 DMAs (two ~0.5-1MB strided transfers each for T/inT
    halves) so SP descriptor-gen time stays off the critical path; the ACT
    exp table is pre-warmed during the DMAs.
"""

import sys
import numpy as np

for _p in ("/opt/trn_rl_repo",):
    if _p not in sys.path:
        sys.path.insert(0, _p)

B = 512
F = 1024
K = 100
D = 5
KD = K * D  # 500
NCORES = 8
JPC = B // NCORES  # 64 output rows per core
NCHUNK = 4  # kd chunks of 125
CHUNK = KD // NCHUNK  # 125
KPC = K // NCHUNK  # 25 k's per chunk
NBLK = 5  # col block-groups computed per core (k = 0..4)
W = NBLK * JPC  # 320 pairwise columns per core
NEX = 3  # exchanged colsum groups (k = 1, 2, 3)
CEX = NEX * JPC  # 192 exchanged columns (local cols 64..256)

_NC_CACHE = {}


def build_nc():
    import contextlib

    import concourse.bass as bass
    import concourse.bacc as bacc
    import concourse.mybir as mybir
    from concourse.tile import TileContext

    nc = bacc.Bacc(None, target_bir_lowering=False, debug=True)

    inT = nc.declare_dram_parameter("inT", [F, W], mybir.dt.float16, isOutput=False)
    Tm = nc.declare_dram_parameter("Tm", [F, KD], mybir.dt.float16, isOutput=False)
    # dmat[5m+d, m] = 2.0 (d-sum of 2*relu), dmat[5m+d, 32+m] = 1.0 (S row sums)
    dmat = nc.declare_dram_parameter(
        "dmat", [CHUNK, 64], mybir.dt.float16, isOutput=False
    )
    negI = nc.declare_dram_parameter("negI", [128, 128], mybir.dt.float16, isOutput=False)
    rowsum = nc.declare_dram_parameter("rowsum", [128, JPC], mybir.dt.float32, isOutput=True)
    colout = nc.declare_dram_parameter(
        "colout", [128, JPC + CEX], mybir.dt.float32, isOutput=True
    )

    with TileContext(nc) as tc:
        with tc.tile_pool(name="persist", bufs=1) as pp:
            T_sb = pp.tile([128, 8 * KD], mybir.dt.float16, name="T_sb")
            inT_sb = pp.tile([128, 8 * W], mybir.dt.float16, name="inT_sb")
            dmat_sb = pp.tile([CHUNK, 64], mybir.dt.float16, name="dmat_sb")
            S16_sb = pp.tile([128, W], mybir.dt.float16, name="S16_sb")
            negSj_sb = pp.tile([128, JPC], mybir.dt.float32, name="negSj_sb")
            colacc_sb = pp.tile([128, JPC + CEX], mybir.dt.float32, name="colacc_sb")
            negI_sb = pp.tile([128, 128], mybir.dt.float16, name="negI_sb")
            xT_sb = pp.tile([128, NCHUNK * W], mybir.dt.float16, name="xT_sb")
            # f32 upcasts of xT columns 0..JPC (tensor_scalar per-partition
            # scalars must be f32). Upcast from the fp16 xT so the diagonal
            # |x - x| stays exactly zero.
            xTj_sb = pp.tile([128, NCHUNK * JPC], mybir.dt.float32, name="xTj_sb")
            raw_sb = pp.tile([128, JPC], mybir.dt.float32, name="raw_sb")

            warm_sb = pp.tile([1, 1], mybir.dt.float32, name="warm_sb")
            # Static rings: every (j, chunk) gets its own ab tile and every
            # j its own dump slot modulo 8 — cross-iteration WAW deps are
            # either absent (ab) or satisfied 8 iterations early (dump).
            ab_ring = [
                pp.tile([CHUNK, W - (t // NCHUNK)], mybir.dt.float16, name=f"ab{t}")
                for t in range(JPC * NCHUNK)
            ]
            NDUMP = 8
            dump_ring = [
                pp.tile([128, W], mybir.dt.float16, name=f"dump{t}")
                for t in range(NDUMP)
            ]

            # --- load inputs: 2 halves each of T/inT so matmuls can start
            # after the first halves land, in 6 total strided DMAs ---
            # the cost of a DMA is charged to its issuing queue (SP/ACT/
            # Pool are the only DMA-capable queues): T halves run in
            # parallel on SP+ACT, inT halves on Pool, so the PE matmuls
            # start ~0.9us earlier
            dma_engines = [nc.sync, nc.scalar]
            for h in range(2):
                dma_engines[h].dma_start(
                    out=T_sb[:, h * 4 * KD : (h + 1) * 4 * KD].rearrange(
                        "p (t k) -> p t k", t=4
                    ),
                    in_=Tm[h * 512 : (h + 1) * 512, :].rearrange(
                        "(t p) k -> p t k", t=4
                    ),
                )
            for h in range(2):
                nc.gpsimd.dma_start(
                    out=inT_sb[:, h * 4 * W : (h + 1) * 4 * W].rearrange(
                        "p (t w) -> p t w", t=4
                    ),
                    in_=inT[h * 512 : (h + 1) * 512, :].rearrange(
                        "(t p) w -> p t w", t=4
                    ),
                )
            nc.sync.dma_start(out=dmat_sb[:, :], in_=dmat[:, :])
            nc.sync.dma_start(out=negI_sb[:, :], in_=negI[:, :])
            # zero the Pool-side column-sum accumulator while DMAs run
            nc.gpsimd.memset(colacc_sb[:, :], 0.0)
            # warm the ACT exp table while the xT matmuls run (~1.3us)
            nc.vector.memset(warm_sb[:, :], 0.0)
            nc.scalar.activation(
                warm_sb[:, :], warm_sb[:, :], mybir.ActivationFunctionType.Exp
            )

            with tc.tile_pool(name="xtps", bufs=3, space="PSUM") as xtps:
                # --- xT chunks: xT[kd, i] via PE over f tiles; the S-row-sum
                # quadrant matmul for each chunk is emitted right after its
                # copy so the S16/negSj chain never serializes at the end ---
                S_ps = xtps.tile([128, W], mybir.dt.float32, name="S_ps", bufs=1)
                for c in range(NCHUNK):
                    xt_ps = xtps.tile([CHUNK, W], mybir.dt.float32, name="xt_ps")
                    for t in range(8):
                        nc.tensor.matmul(
                            xt_ps[:, :],
                            T_sb[:, t * KD + c * CHUNK : t * KD + (c + 1) * CHUNK],
                            inT_sb[:, t * W : (t + 1) * W],
                            start=(t == 0),
                            stop=(t == 7),
                        )
                    # alternate the PSUM->SBUF fp16 copies between DVE and
                    # ACT so the input stage drains faster
                    if c % 2 == 0:
                        nc.vector.tensor_copy(
                            xT_sb[0:CHUNK, c * W : (c + 1) * W], xt_ps[:, :]
                        )
                    else:
                        nc.scalar.copy(xT_sb[0:CHUNK, c * W : (c + 1) * W], xt_ps[:, :])
                    nc.vector.tensor_copy(
                        xTj_sb[0:CHUNK, c * JPC : (c + 1) * JPC],
                        xT_sb[0:CHUNK, c * W : c * W + JPC],
                    )
                    # S[k, i] = sum_d x[i,k,d] at partitions 32c+m
                    nc.tensor.matmul(
                        S_ps[32 * c : 32 * c + 32, :],
                        dmat_sb[:, 32:64],
                        xT_sb[0:CHUNK, c * W : (c + 1) * W],
                        start=True,
                        stop=True,
                        tile_position=(0, 32 * c),
                        skip_group_check=True,
                    )
                nc.scalar.copy(S16_sb[:, :], S_ps[:, :])
                # exp bias column: -S_j, upcast from the SAME fp16 S16 the
                # negI matmul reads so the diagonal cancels exactly
                nc.vector.tensor_scalar(
                    negSj_sb[:, :],
                    S16_sb[:, 0:JPC],
                    -1.0,
                    0.0,
                    mybir.AluOpType.mult,
                    mybir.AluOpType.bypass,
                )

            mainps_es = contextlib.ExitStack()
            mainps = mainps_es.enter_context(
                tc.tile_pool(name="mainps", bufs=1, space="PSUM")
            )
            NDIST = 6
            dist_bufs = [
                mainps.tile([128, W], mybir.dt.float32, name=f"dist{i}")
                for i in range(NDIST)
            ]

            # --- main loop over output rows ---
            # Row j only computes columns i >= j (ragged upper triangle):
            # the diagonal block's lower-triangle contributions come from the
            # colacc by symmetry (minus the double-counted self term 1.0,
            # subtracted on the host).
            CEND = JPC + CEX  # colacc covers cols 0..256
            for j in range(JPC):
                dist = dist_bufs[j % NDIST]
                # dist = -S[k, i] (also absorbs the WAR wait vs the ACT exp
                # that last read this dist buffer)
                nc.tensor.matmul(
                    dist[:, j:W],
                    negI_sb[:, :],
                    S16_sb[:, j:W],
                    start=True,
                    stop=False,
                    skip_group_check=True,
                )
                for c in range(NCHUNK):
                    ab = ab_ring[j * NCHUNK + c]
                    # ab = relu(xT[:, i] - xT[:, j]) : (in - s1) max 0.0
                    # (const scalar2 keeps the second DVE read port free so
                    # the 4x perf mode applies)
                    nc.vector.tensor_scalar(
                        ab[:, :],
                        xT_sb[0:CHUNK, c * W + j : (c + 1) * W],
                        xTj_sb[0:CHUNK, c * JPC + j : c * JPC + j + 1],
                        0.0,
                        mybir.AluOpType.subtract,
                        mybir.AluOpType.max,
                    )
                    # dist[32c+m, :] += 2 * sum_d ab[5m+d, :]
                    nc.tensor.matmul(
                        dist[32 * c : 32 * c + 32, j:W],
                        dmat_sb[:, 0:32],
                        ab[:, :],
                        start=False,
                        stop=(c == NCHUNK - 1),
                        tile_position=(0, 32 * c),
                        skip_group_check=True,
                    )
                # dump = exp(-dist - S_j) = exp(-L1(i,j)) fp16;
                # accum_out gives the row sums over cols j..320
                nc.scalar.activation(
                    dump_ring[j % NDUMP][:, j:W],
                    dist[:, j:W],
                    mybir.ActivationFunctionType.Exp,
                    bias=negSj_sb[:, j : j + 1],
                    scale=-1.0,
                    accum_out=raw_sb[:, j : j + 1],
                )
                if j == 31:
                    # first half of the row sums is final: overlap its DMA
                    nc.sync.dma_start(out=rowsum[:, 0:32], in_=raw_sb[:, 0:32])
                # column sums on the otherwise-idle Pool (GPSIMD) engine:
                # colacc += dump[:, j:256] (f32 accumulator in SBUF) — the
                # diag block part doubles as the lower-triangle row sums
                nc.gpsimd.tensor_tensor(
                    colacc_sb[:, j:CEND],
                    colacc_sb[:, j:CEND],
                    dump_ring[j % NDUMP][:, j:CEND],
                    mybir.AluOpType.add,
                )

            mainps_es.close()
            nc.sync.dma_start(out=rowsum[:, 32:JPC], in_=raw_sb[:, 32:JPC])
            nc.gpsimd.dma_start(out=colout[:, :], in_=colacc_sb[:, :])

    nc.finalize()
    return nc


def _aux_consts():
    dm = np.zeros([CHUNK, 64], dtype=np.float16)
    for m in range(KPC):
        dm[5 * m : 5 * m + 5, m] = 2.0
        dm[5 * m : 5 * m + 5, 32 + m] = 1.0
    negI = (-np.eye(128)).astype(np.float16)
    return dm, negI


def make_in_maps(inputs, T):
    f16 = np.float16
    Tm = np.asarray(T, dtype=np.float32).astype(f16)
    dm, negI = _aux_consts()
    in_maps = []
    x = np.asarray(inputs, dtype=np.float32)
    for c in range(NCORES):
        rolled = np.roll(x, -JPC * c, axis=0)[0:W, :]
        inTc = np.ascontiguousarray(rolled.T).astype(f16)
        in_maps.append(
            {
                "inT": inTc,
                "Tm": Tm,
                "dmat": dm,
                "negI": negI,
            }
        )
    return in_maps


def assemble_output(results):
    out = np.zeros([B, K], dtype=np.float32)
    # own row sums: raw[32c+m, j] -> out[64q+j, 25c+m]
    for q in range(NCORES):
        raw = np.asarray(results[q]["rowsum"], dtype=np.float32)  # [128, JPC]
        for cc in range(NCHUNK):
            out[JPC * q : JPC * (q + 1), KPC * cc : KPC * (cc + 1)] = raw[
                32 * cc : 32 * cc + KPC, :
            ].T
    # column sums: core b's group k serves rows of core b+k. k=0 is the own
    # diag block (lower triangle by symmetry; subtract the double-counted
    # self term exp(0)=1), k=1..3 are the exchanged off-diag groups.
    for b in range(NCORES):
        col = np.asarray(results[b]["colout"], dtype=np.float32)  # [128, JPC+CEX]
        for k in range(0, NEX + 1):
            q = (b + k) % NCORES
            blk = col[:, JPC * k : JPC * (k + 1)]  # [128, JPC]
            for cc in range(NCHUNK):
                out[JPC * q : JPC * (q + 1), KPC * cc : KPC * (cc + 1)] += blk[
                    32 * cc : 32 * cc + KPC, :
                ].T
    out -= 1.0
    return out


def kernel(inputs, T):
    from concourse.bass_utils import run_bass_kernel_spmd

    if "nc" not in _NC_CACHE:
        _NC_CACHE["nc"] = build_nc()
    nc = _NC_CACHE["nc"]
    in_maps = make_in_maps(inputs, T)
    res = run_bass_kernel_spmd(nc, in_maps, list(range(NCORES)))
    return assemble_output(res.results)


if __name__ == "__main__":
    sys.path.insert(0, "/root/problem")
    from reference import setup_inputs, reference

    inputs = setup_inputs()
    expected = np.asarray(reference(**inputs))
    actual = kernel(**{k: np.asarray(v) for k, v in inputs.items()})
    err = np.abs(actual - expected)
    rel = np.linalg.norm(actual - expected) / np.linalg.norm(expected)
    print(f"max abs err: {err.max():.3e}")
    print(f"Relative error: {rel:.3e}")


# revision 12
# speedup vs baseline: 1.0379x; 1.0379x over previous
"""
MinibatchDiscrimination kernel for 8x TRN2 NeuronCores (Bass/Tile).

Math:  x = inputs @ T  -> [B, K, D] with B=512, K=100, D=5
       out[i,k] = sum_j exp(-sum_d |x[i,k,d]-x[j,k,d]|)

Strategy — symmetric block-tournament over the pairwise matrix:

  The B x B pairwise matrix is tiled into 8x8 blocks of 64x64 (one row-group
  per core). Each unordered block-pair only needs computing once: from one
  computed block, ROW sums come from the ACT accumulator and COLUMN sums
  (= row sums of the transposed block, by symmetry of the L1 distance) come
  from a PE identity-matmul accumulation over the exp tiles. Core c computes
  blocks (c, c+k) for k=0..4 (mod 8, W=320 columns of its rolled copy):

    - diag block (k=0): row sums only (colsum would double-count by symmetry)
    - k=1,2,3: row sums kept locally + column sums exchanged to core c+k
      (exchange happens on the host during output assembly)
    - k=4: row sums only; the mirror pair {c, c+4} is computed independently
      by core c+4 as ITS k=4 block (distance-4 blocks are duplicated so the
      SPMD program stays identical across cores)

  Row j of core q then receives: own row sums (col-groups q..q+4) plus
  exchanged column sums from cores q-1, q-2, q-3 — all 8 groups exactly once.

Per core c of 8 (rolled by 64c so the program is SPMD-identical):
  - xT[kd, i] = sum_f T[f, kd] * inT[f, i] on PE (4 chunks of 125 kd), i<320.
  - Per output row j in 0..63:
      ab_c[p, i] = |xT_c[p, i] - xT_c[p, j]|   (DVE tensor_scalar
                   (subtract, abs_max vs 0.0) — fp16, 4x perf mode;
                   the per-partition scalar is an f32 upcast of the fp16 xT
                   column so the diagonal is exactly 0)
      dist[32c+m, :] = sum_d ab[5m+d, :]       (PE d-sum matmul with a
                   0/1 block matrix, col-tiled per chunk — no S-term or
                   negI matmul needed since abs values sum directly)
      dump[:, :]  = exp(-dist), fp16 -> SBUF   (ACT, accum_out gives the
                   row sums over all 320 cols in one pass)
      colacc     += dump[:, 64:256]            (PE identity matmul
                   accumulating in PSUM across all 64 j — the k=1,2,3
                   column sums, emitted 2 iterations late to pipeline)
  - dist row p=32c+m holds k=25c+m (m<25); host transposes/reassembles and
    adds the exchanged column-sum blocks.

  Hardware notes (CoreSim cost model, validated on TRN2 previously):
  - Steady state is ACT-bound: exp main pass 0.833*320+185 = 452ns plus the
    fixed 287ns accumulator-read = ~739ns/row; DVE 4x tensor_scalars at
    143.8ns = 575ns/row and PE 4 d-sums + colacc = 613ns/row overlap under
    it. 64 rows -> ~47us steady.
  - ab/dump tiles are STATIC rings sized to the whole loop (256 ab tiles,
    ~160KB of SBUF) so there are no cross-iteration WAW deps at all: DVE
    instructions carry no waits in steady state (the baseline lost ~10us+
    to 242 same-engine WAW EventSemaphores from rotating small rings).
  - Inputs land in 4 DMAs (two ~0.5-1MB strided transfers each for T/inT
    halves) so SP descriptor-gen time stays off the critical path; the ACT
    exp table is pre-warmed during the DMAs.
"""

import sys
import numpy as np

for _p in ("/opt/trn_rl_repo",):
    if _p not in sys.path:
        sys.path.insert(0, _p)

B = 512
F = 1024
K = 100
D = 5
KD = K * D  # 500
NCORES = 8
JPC = B // NCORES  # 64 output rows per core
NCHUNK = 4  # kd chunks of 125
CHUNK = KD // NCHUNK  # 125
KPC = K // NCHUNK  # 25 k's per chunk
NBLK = 5  # col block-groups computed per core (k = 0..4)
W = NBLK * JPC  # 320 pairwise columns per core
NEX = 3  # exchanged colsum groups (k = 1, 2, 3)
CEX = NEX * JPC  # 192 exchanged columns (local cols 64..256)

_NC_CACHE = {}


def build_nc():
    import contextlib

    import concourse.bass as bass
    import concourse.bacc as bacc
    import concourse.mybir as mybir
    from concourse.tile import TileContext

    nc = bacc.Bacc(None, target_bir_lowering=False, debug=True)

    inT = nc.declare_dram_parameter("inT", [F, W], mybir.dt.float16, isOutput=False)
    Tm = nc.declare_dram_parameter("Tm", [F, KD], mybir.dt.float16, isOutput=False)
    # dmat[5m+d, m] = 2.0 (d-sum of 2*relu), dmat[5m+d, 32+m] = 1.0 (S row sums)
    dmat = nc.declare_dram_parameter(
        "dmat", [CHUNK, 64], mybir.dt.float16, isOutput=False
    )
    negI = nc.declare_dram_parameter("negI", [128, 128], mybir.dt.float16, isOutput=False)
    rowsum = nc.declare_dram_parameter("rowsum", [128, JPC], mybir.dt.float32, isOutput=True)
    colout = nc.declare_dram_parameter(
        "colout", [128, JPC + CEX], mybir.dt.float32, isOutput=True
    )

    with TileContext(nc) as tc:
        with tc.tile_pool(name="persist", bufs=1) as pp:
            T_sb = pp.tile([128, 8 * KD], mybir.dt.float16, name="T_sb")
            inT_sb = pp.tile([128, 8 * W], mybir.dt.float16, name="inT_sb")
            dmat_sb = pp.tile([CHUNK, 64], mybir.dt.float16, name="dmat_sb")
            S16_sb = pp.tile([128, W], mybir.dt.float16, name="S16_sb")
            negSj_sb = pp.tile([128, JPC], mybir.dt.float32, name="negSj_sb")
            colacc_sb = pp.tile([128, JPC + CEX], mybir.dt.float32, name="colacc_sb")
            negI_sb = pp.tile([128, 128], mybir.dt.float16, name="negI_sb")
            xT_sb = pp.tile([128, NCHUNK * W], mybir.dt.float16, name="xT_sb")
            # f32 upcasts of xT columns 0..JPC (tensor_scalar per-partition
            # scalars must be f32). Upcast from the fp16 xT so the diagonal
            # |x - x| stays exactly zero.
            xTj_sb = pp.tile([128, NCHUNK * JPC], mybir.dt.float32, name="xTj_sb")
            raw_sb = pp.tile([128, JPC], mybir.dt.float32, name="raw_sb")

            warm_sb = pp.tile([1, 1], mybir.dt.float32, name="warm_sb")
            # Static rings: every (j, chunk) gets its own ab tile and every
            # j its own dump slot modulo 8 — cross-iteration WAW deps are
            # either absent (ab) or satisfied 8 iterations early (dump).
            ab_ring = [
                pp.tile([CHUNK, W - (t // NCHUNK)], mybir.dt.float16, name=f"ab{t}")
                for t in range(JPC * NCHUNK)
            ]
            NDUMP = 8
            dump_ring = [
                pp.tile([128, W], mybir.dt.float16, name=f"dump{t}")
                for t in range(NDUMP)
            ]

            # --- load inputs: 2 halves each of T/inT so matmuls can start
            # after the first halves land, in 6 total strided DMAs ---
            # the cost of a DMA is charged to its issuing queue (SP/ACT/
            # Pool are the only DMA-capable queues): T halves run in
            # parallel on SP+ACT, inT halves on Pool, so the PE matmuls
            # start ~0.9us earlier
            def dma_T(e, h):
                e.dma_start(
                    out=T_sb[:, h * 4 * KD : (h + 1) * 4 * KD].rearrange(
                        "p (t k) -> p t k", t=4
                    ),
                    in_=Tm[h * 512 : (h + 1) * 512, :].rearrange(
                        "(t p) k -> p t k", t=4
                    ),
                )

            def dma_inT(e, h):
                e.dma_start(
                    out=inT_sb[:, h * 4 * W : (h + 1) * 4 * W].rearrange(
                        "p (t w) -> p t w", t=4
                    ),
                    in_=inT[h * 512 : (h + 1) * 512, :].rearrange(
                        "(t p) w -> p t w", t=4
                    ),
                )

            dma_T(nc.sync, 0)
            dma_inT(nc.gpsimd, 0)
            dma_T(nc.gpsimd, 1)
            dma_inT(nc.scalar, 1)
            nc.sync.dma_start(out=dmat_sb[:, :], in_=dmat[:, :])
            nc.sync.dma_start(out=negI_sb[:, :], in_=negI[:, :])
            # zero the Pool-side column-sum accumulator while DMAs run
            nc.gpsimd.memset(colacc_sb[:, :], 0.0)
            # warm the ACT exp table while the xT matmuls run (~1.3us)
            nc.vector.memset(warm_sb[:, :], 0.0)
            nc.scalar.activation(
                warm_sb[:, :], warm_sb[:, :], mybir.ActivationFunctionType.Exp
            )

            with tc.tile_pool(name="xtps", bufs=3, space="PSUM") as xtps:
                # "gate" matmuls: each absorbs exactly one input-DMA (or
                # const-DMA) semaphore inline, so no real matmul carries two
                # waits — a 2-wait matmul gets a separate EventSemaphore on
                # the PE queue which resets the p-state ramp clock, dropping
                # the whole input stage to half clock
                gate_srcs = [
                    T_sb[:, 0:64],
                    inT_sb[:, 0:64],
                    T_sb[:, 4 * KD : 4 * KD + 64],
                    inT_sb[:, 4 * W : 4 * W + 64],
                    dmat_sb[:, 0:64],
                    negI_sb[:, 0:64],
                ]
                g_ps0 = xtps.tile([128, 64], mybir.dt.float32, name="gate_a", bufs=1)
                g_ps1 = xtps.tile([128, 64], mybir.dt.float32, name="gate_b", bufs=1)
                for gi, gsrc in enumerate(gate_srcs):
                    g_ps = g_ps0 if gi < 4 else g_ps1
                    q = gi % 4
                    nc.tensor.matmul(
                        g_ps[32 * q : 32 * q + 32, :],
                        gsrc[:, 0:32],
                        gsrc[:, 0:64],
                        start=True,
                        stop=True,
                        tile_position=(0, 32 * q),
                        skip_group_check=True,
                    )
                # --- xT chunks: xT[kd, i] via PE over f tiles; the S-row-sum
                # quadrant matmul for each chunk is emitted right after its
                # copy so the S16/negSj chain never serializes at the end ---
                S_ps = xtps.tile([128, W], mybir.dt.float32, name="S_ps", bufs=1)
                for c in range(NCHUNK):
                    xt_ps = xtps.tile([CHUNK, W], mybir.dt.float32, name="xt_ps")
                    for t in range(8):
                        nc.tensor.matmul(
                            xt_ps[:, :],
                            T_sb[:, t * KD + c * CHUNK : t * KD + (c + 1) * CHUNK],
                            inT_sb[:, t * W : (t + 1) * W],
                            start=(t == 0),
                            stop=(t == 7),
                        )
                    # alternate the PSUM->SBUF fp16 copies between DVE and
                    # ACT so the input stage drains faster
                    if c % 2 == 0:
                        nc.vector.tensor_copy(
                            xT_sb[0:CHUNK, c * W : (c + 1) * W], xt_ps[:, :]
                        )
                    else:
                        nc.scalar.copy(xT_sb[0:CHUNK, c * W : (c + 1) * W], xt_ps[:, :])
                    nc.vector.tensor_copy(
                        xTj_sb[0:CHUNK, c * JPC : (c + 1) * JPC],
                        xT_sb[0:CHUNK, c * W : c * W + JPC],
                    )
                    # S[k, i] = sum_d x[i,k,d] at partitions 32c+m
                    nc.tensor.matmul(
                        S_ps[32 * c : 32 * c + 32, :],
                        dmat_sb[:, 32:64],
                        xT_sb[0:CHUNK, c * W : (c + 1) * W],
                        start=True,
                        stop=True,
                        tile_position=(0, 32 * c),
                        skip_group_check=True,
                    )
                nc.scalar.copy(S16_sb[:, :], S_ps[:, :])
                # exp bias column: -S_j, upcast from the SAME fp16 S16 the
                # negI matmul reads so the diagonal cancels exactly
                nc.vector.tensor_scalar(
                    negSj_sb[:, :],
                    S16_sb[:, 0:JPC],
                    -1.0,
                    0.0,
                    mybir.AluOpType.mult,
                    mybir.AluOpType.bypass,
                )

            mainps_es = contextlib.ExitStack()
            mainps = mainps_es.enter_context(
                tc.tile_pool(name="mainps", bufs=1, space="PSUM")
            )
            NDIST = 6
            dist_bufs = [
                mainps.tile([128, W], mybir.dt.float32, name=f"dist{i}")
                for i in range(NDIST)
            ]

            # --- main loop over output rows ---
            # Row j only computes columns i >= j (ragged upper triangle):
            # the diagonal block's lower-triangle contributions come from the
            # colacc by symmetry (minus the double-counted self term 1.0,
            # subtracted on the host).
            CEND = JPC + CEX  # colacc covers cols 0..256
            for j in range(JPC):
                dist = dist_bufs[j % NDIST]
                # dist = -S[k, i] (also absorbs the WAR wait vs the ACT exp
                # that last read this dist buffer)
                nc.tensor.matmul(
                    dist[:, j:W],
                    negI_sb[:, :],
                    S16_sb[:, j:W],
                    start=True,
                    stop=False,
                    skip_group_check=True,
                )
                for c in range(NCHUNK):
                    ab = ab_ring[j * NCHUNK + c]
                    # ab = relu(xT[:, i] - xT[:, j]) : (in - s1) max 0.0
                    # (const scalar2 keeps the second DVE read port free so
                    # the 4x perf mode applies)
                    nc.vector.tensor_scalar(
                        ab[:, :],
                        xT_sb[0:CHUNK, c * W + j : (c + 1) * W],
                        xTj_sb[0:CHUNK, c * JPC + j : c * JPC + j + 1],
                        0.0,
                        mybir.AluOpType.subtract,
                        mybir.AluOpType.max,
                    )
                    # dist[32c+m, :] += 2 * sum_d ab[5m+d, :]
                    nc.tensor.matmul(
                        dist[32 * c : 32 * c + 32, j:W],
                        dmat_sb[:, 0:32],
                        ab[:, :],
                        start=False,
                        stop=(c == NCHUNK - 1),
                        tile_position=(0, 32 * c),
                        skip_group_check=True,
                    )
                # dump = exp(-dist - S_j) = exp(-L1(i,j)) fp16;
                # accum_out gives the row sums over cols j..320
                nc.scalar.activation(
                    dump_ring[j % NDUMP][:, j:W],
                    dist[:, j:W],
                    mybir.ActivationFunctionType.Exp,
                    bias=negSj_sb[:, j : j + 1],
                    scale=-1.0,
                    accum_out=raw_sb[:, j : j + 1],
                )
                if j == 31:
                    # first half of the row sums is final: overlap its DMA
                    nc.sync.dma_start(out=rowsum[:, 0:32], in_=raw_sb[:, 0:32])
                # column sums on the otherwise-idle Pool (GPSIMD) engine:
                # colacc += dump[:, j:256] (f32 accumulator in SBUF) — the
                # diag block part doubles as the lower-triangle row sums
                nc.gpsimd.tensor_tensor(
                    colacc_sb[:, j:CEND],
                    colacc_sb[:, j:CEND],
                    dump_ring[j % NDUMP][:, j:CEND],
                    mybir.AluOpType.add,
                )

            mainps_es.close()
            nc.sync.dma_start(out=rowsum[:, 32:JPC], in_=raw_sb[:, 32:JPC])
            nc.gpsimd.dma_start(out=colout[:, :], in_=colacc_sb[:, :])

    nc.finalize()
    return nc


def _aux_consts():
    dm = np.zeros([CHUNK, 64], dtype=np.float16)
    for m in range(KPC):
        dm[5 * m : 5 * m + 5, m] = 2.0
        dm[5 * m : 5 * m + 5, 32 + m] = 1.0
    negI = (-np.eye(128)).astype(np.float16)
    return dm, negI


def make_in_maps(inputs, T):
    f16 = np.float16
    Tm = np.asarray(T, dtype=np.float32).astype(f16)
    dm, negI = _aux_consts()
    in_maps = []
    x = np.asarray(inputs, dtype=np.float32)
    for c in range(NCORES):
        rolled = np.roll(x, -JPC * c, axis=0)[0:W, :]
        inTc = np.ascontiguousarray(rolled.T).astype(f16)
        in_maps.append(
            {
                "inT": inTc,
                "Tm": Tm,
                "dmat": dm,
                "negI": negI,
            }
        )
    return in_maps


def assemble_output(results):
    out = np.zeros([B, K], dtype=np.float32)
    # own row sums: raw[32c+m, j] -> out[64q+j, 25c+m]
    for q in range(NCORES):
        raw = np.asarray(results[q]["rowsum"], dtype=np.float32)  # [128, JPC]
        for cc in range(NCHUNK):
            out[JPC * q : JPC * (q + 1), KPC * cc : KPC * (cc + 1)] = raw[
                32 * cc : 32 * cc + KPC, :
            ].T
    # column sums: core b's group k serves rows of core b+k. k=0 is the own
    # diag block (lower triangle by symmetry; subtract the double-counted
    # self term exp(0)=1), k=1..3 are the exchanged off-diag groups.
    for b in range(NCORES):
        col = np.asarray(results[b]["colout"], dtype=np.float32)  # [128, JPC+CEX]
        for k in range(0, NEX + 1):
            q = (b + k) % NCORES
            blk = col[:, JPC * k : JPC * (k + 1)]  # [128, JPC]
            for cc in range(NCHUNK):
                out[JPC * q : JPC * (q + 1), KPC * cc : KPC * (cc + 1)] += blk[
                    32 * cc : 32 * cc + KPC, :
                ].T
    out -= 1.0
    return out


def kernel(inputs, T):
    from concourse.bass_utils import run_bass_kernel_spmd

    if "nc" not in _NC_CACHE:
        _NC_CACHE["nc"] = build_nc()
    nc = _NC_CACHE["nc"]
    in_maps = make_in_maps(inputs, T)
    res = run_bass_kernel_spmd(nc, in_maps, list(range(NCORES)))
    return assemble_output(res.results)


if __name__ == "__main__":
    sys.path.insert(0, "/root/problem")
    from reference import setup_inputs, reference

    inputs = setup_inputs()
    expected = np.asarray(reference(**inputs))
    actual = kernel(**{k: np.asarray(v) for k, v in inputs.items()})
    err = np.abs(actual - expected)
    rel = np.linalg.norm(actual - expected) / np.linalg.norm(expected)
    print(f"max abs err: {err.max():.3e}")
    print(f"Relative error: {rel:.3e}")


# revision 13
# speedup vs baseline: 1.0550x; 1.0164x over previous
"""
MinibatchDiscrimination kernel for 8x TRN2 NeuronCores (Bass/Tile).

Math:  x = inputs @ T  -> [B, K, D] with B=512, K=100, D=5
       out[i,k] = sum_j exp(-sum_d |x[i,k,d]-x[j,k,d]|)

Strategy — symmetric block-tournament over the pairwise matrix:

  The B x B pairwise matrix is tiled into 8x8 blocks of 64x64 (one row-group
  per core). Each unordered block-pair only needs computing once: from one
  computed block, ROW sums come from the ACT accumulator and COLUMN sums
  (= row sums of the transposed block, by symmetry of the L1 distance) come
  from a PE identity-matmul accumulation over the exp tiles. Core c computes
  blocks (c, c+k) for k=0..4 (mod 8, W=320 columns of its rolled copy):

    - diag block (k=0): row sums only (colsum would double-count by symmetry)
    - k=1,2,3: row sums kept locally + column sums exchanged to core c+k
      (exchange happens on the host during output assembly)
    - k=4: row sums only; the mirror pair {c, c+4} is computed independently
      by core c+4 as ITS k=4 block (distance-4 blocks are duplicated so the
      SPMD program stays identical across cores)

  Row j of core q then receives: own row sums (col-groups q..q+4) plus
  exchanged column sums from cores q-1, q-2, q-3 — all 8 groups exactly once.

Per core c of 8 (rolled by 64c so the program is SPMD-identical):
  - xT[kd, i] = sum_f T[f, kd] * inT[f, i] on PE (4 chunks of 125 kd), i<320.
  - Per output row j in 0..63:
      ab_c[p, i] = |xT_c[p, i] - xT_c[p, j]|   (DVE tensor_scalar
                   (subtract, abs_max vs 0.0) — fp16, 4x perf mode;
                   the per-partition scalar is an f32 upcast of the fp16 xT
                   column so the diagonal is exactly 0)
      dist[32c+m, :] = sum_d ab[5m+d, :]       (PE d-sum matmul with a
                   0/1 block matrix, col-tiled per chunk — no S-term or
                   negI matmul needed since abs values sum directly)
      dump[:, :]  = exp(-dist), fp16 -> SBUF   (ACT, accum_out gives the
                   row sums over all 320 cols in one pass)
      colacc     += dump[:, 64:256]            (PE identity matmul
                   accumulating in PSUM across all 64 j — the k=1,2,3
                   column sums, emitted 2 iterations late to pipeline)
  - dist row p=32c+m holds k=25c+m (m<25); host transposes/reassembles and
    adds the exchanged column-sum blocks.

  Hardware notes (CoreSim cost model, validated on TRN2 previously):
  - Steady state is ACT-bound: exp main pass 0.833*320+185 = 452ns plus the
    fixed 287ns accumulator-read = ~739ns/row; DVE 4x tensor_scalars at
    143.8ns = 575ns/row and PE 4 d-sums + colacc = 613ns/row overlap under
    it. 64 rows -> ~47us steady.
  - ab/dump tiles are STATIC rings sized to the whole loop (256 ab tiles,
    ~160KB of SBUF) so there are no cross-iteration WAW deps at all: DVE
    instructions carry no waits in steady state (the baseline lost ~10us+
    to 242 same-engine WAW EventSemaphores from rotating small rings).
  - Inputs land in 4 DMAs (two ~0.5-1MB strided transfers each for T/inT
    halves) so SP descriptor-gen time stays off the critical path; the ACT
    exp table is pre-warmed during the DMAs.
"""

import sys
import numpy as np

for _p in ("/opt/trn_rl_repo",):
    if _p not in sys.path:
        sys.path.insert(0, _p)

B = 512
F = 1024
K = 100
D = 5
KD = K * D  # 500
NCORES = 8
JPC = B // NCORES  # 64 output rows per core
NCHUNK = 4  # kd chunks of 125
CHUNK = KD // NCHUNK  # 125
KPC = K // NCHUNK  # 25 k's per chunk
NBLK = 5  # col block-groups computed per core (k = 0..4)
W = NBLK * JPC  # 320 pairwise columns per core
NEX = 3  # exchanged colsum groups (k = 1, 2, 3)
CEX = NEX * JPC  # 192 exchanged columns (local cols 64..256)
D4HI = 4 * JPC  # 256: start of the distance-4 high-half cols (rows 32..64 of c+4)
D4LO = D4HI + 32  # 288: start of the distance-4 low-half cols (rows 0..32 of c+4)
NCOL = D4LO  # 288 columns exported in colout

_NC_CACHE = {}


def build_nc():
    import contextlib

    import concourse.bass as bass
    import concourse.bacc as bacc
    import concourse.mybir as mybir
    from concourse.tile import TileContext

    nc = bacc.Bacc(None, target_bir_lowering=False, debug=True)

    inT = nc.declare_dram_parameter("inT", [F, W], mybir.dt.float16, isOutput=False)
    Tm = nc.declare_dram_parameter("Tm", [F, KD], mybir.dt.float16, isOutput=False)
    # dmat[5m+d, m] = 2.0 (d-sum of 2*relu), dmat[5m+d, 32+m] = 1.0 (S row sums)
    dmat = nc.declare_dram_parameter(
        "dmat", [CHUNK, 64], mybir.dt.float16, isOutput=False
    )
    negI = nc.declare_dram_parameter("negI", [128, 128], mybir.dt.float16, isOutput=False)
    rowsum = nc.declare_dram_parameter("rowsum", [128, JPC], mybir.dt.float32, isOutput=True)
    colout = nc.declare_dram_parameter(
        "colout", [128, NCOL], mybir.dt.float32, isOutput=True
    )

    with TileContext(nc) as tc:
        with tc.tile_pool(name="persist", bufs=1) as pp:
            T_sb = pp.tile([128, 8 * KD], mybir.dt.float16, name="T_sb")
            inT_sb = pp.tile([128, 8 * W], mybir.dt.float16, name="inT_sb")
            dmat_sb = pp.tile([CHUNK, 64], mybir.dt.float16, name="dmat_sb")
            S16_sb = pp.tile([128, W], mybir.dt.float16, name="S16_sb")
            negSj_sb = pp.tile([128, JPC], mybir.dt.float32, name="negSj_sb")
            colacc_sb = pp.tile([128, NCOL], mybir.dt.float32, name="colacc_sb")
            negI_sb = pp.tile([128, 128], mybir.dt.float16, name="negI_sb")
            xT_sb = pp.tile([128, NCHUNK * W], mybir.dt.float16, name="xT_sb")
            # f32 upcasts of xT columns 0..JPC (tensor_scalar per-partition
            # scalars must be f32). Upcast from the fp16 xT so the diagonal
            # |x - x| stays exactly zero.
            xTj_sb = pp.tile([128, NCHUNK * JPC], mybir.dt.float32, name="xTj_sb")
            raw_sb = pp.tile([128, JPC], mybir.dt.float32, name="raw_sb")

            warm_sb = pp.tile([1, 1], mybir.dt.float32, name="warm_sb")
            # Static rings: every (j, chunk) gets its own ab tile and every
            # j its own dump slot modulo 8 — cross-iteration WAW deps are
            # either absent (ab) or satisfied 8 iterations early (dump).
            # row j computes cols j..E(j): E=320 for j<32 (full distance-4
            # block), E=288 for j>=32 (the low half of the distance-4 block
            # is covered by the exchanged colacc of core c-4's rows <32)
            def _erow(j):
                return W if j < 32 else D4LO

            ab_ring = [
                pp.tile(
                    [CHUNK, _erow(t // NCHUNK) - (t // NCHUNK)],
                    mybir.dt.float16,
                    name=f"ab{t}",
                )
                for t in range(JPC * NCHUNK)
            ]
            NDUMP = 8
            dump_ring = [
                pp.tile([128, W], mybir.dt.float16, name=f"dump{t}")
                for t in range(NDUMP)
            ]

            # --- load inputs: 2 halves each of T/inT so matmuls can start
            # after the first halves land, in 6 total strided DMAs ---
            # the cost of a DMA is charged to its issuing queue (SP/ACT/
            # Pool are the only DMA-capable queues): T halves run in
            # parallel on SP+ACT, inT halves on Pool, so the PE matmuls
            # start ~0.9us earlier
            def dma_T(e, h):
                e.dma_start(
                    out=T_sb[:, h * 4 * KD : (h + 1) * 4 * KD].rearrange(
                        "p (t k) -> p t k", t=4
                    ),
                    in_=Tm[h * 512 : (h + 1) * 512, :].rearrange(
                        "(t p) k -> p t k", t=4
                    ),
                )

            def dma_inT(e, h):
                e.dma_start(
                    out=inT_sb[:, h * 4 * W : (h + 1) * 4 * W].rearrange(
                        "p (t w) -> p t w", t=4
                    ),
                    in_=inT[h * 512 : (h + 1) * 512, :].rearrange(
                        "(t p) w -> p t w", t=4
                    ),
                )

            dma_T(nc.sync, 0)
            dma_inT(nc.gpsimd, 0)
            dma_T(nc.gpsimd, 1)
            dma_inT(nc.scalar, 1)
            nc.sync.dma_start(out=dmat_sb[:, :], in_=dmat[:, :])
            nc.sync.dma_start(out=negI_sb[:, :], in_=negI[:, :])
            # zero the Pool-side column-sum accumulator while DMAs run
            nc.gpsimd.memset(colacc_sb[:, :], 0.0)
            # warm the ACT exp table while the xT matmuls run (~1.3us)
            nc.vector.memset(warm_sb[:, :], 0.0)
            nc.scalar.activation(
                warm_sb[:, :], warm_sb[:, :], mybir.ActivationFunctionType.Exp
            )

            with tc.tile_pool(name="xtps", bufs=3, space="PSUM") as xtps:
                # "gate" matmuls: each absorbs exactly one input-DMA (or
                # const-DMA) semaphore inline, so no real matmul carries two
                # waits — a 2-wait matmul gets a separate EventSemaphore on
                # the PE queue which resets the p-state ramp clock, dropping
                # the whole input stage to half clock
                gate_srcs = [
                    T_sb[:, 0:64],
                    inT_sb[:, 0:64],
                    T_sb[:, 4 * KD : 4 * KD + 64],
                    inT_sb[:, 4 * W : 4 * W + 64],
                    dmat_sb[:, 0:64],
                    negI_sb[:, 0:64],
                ]
                g_ps0 = xtps.tile([128, 64], mybir.dt.float32, name="gate_a", bufs=1)
                g_ps1 = xtps.tile([128, 64], mybir.dt.float32, name="gate_b", bufs=1)
                for gi, gsrc in enumerate(gate_srcs):
                    g_ps = g_ps0 if gi < 4 else g_ps1
                    q = gi % 4
                    nc.tensor.matmul(
                        g_ps[32 * q : 32 * q + 32, :],
                        gsrc[:, 0:32],
                        gsrc[:, 0:64],
                        start=True,
                        stop=True,
                        tile_position=(0, 32 * q),
                        skip_group_check=True,
                    )
                # --- xT chunks: xT[kd, i] via PE over f tiles; the S-row-sum
                # quadrant matmul for each chunk is emitted right after its
                # copy so the S16/negSj chain never serializes at the end ---
                S_ps = xtps.tile([128, W], mybir.dt.float32, name="S_ps", bufs=1)
                for c in range(NCHUNK):
                    xt_ps = xtps.tile([CHUNK, W], mybir.dt.float32, name="xt_ps")
                    for t in range(8):
                        nc.tensor.matmul(
                            xt_ps[:, :],
                            T_sb[:, t * KD + c * CHUNK : t * KD + (c + 1) * CHUNK],
                            inT_sb[:, t * W : (t + 1) * W],
                            start=(t == 0),
                            stop=(t == 7),
                        )
                    # alternate the PSUM->SBUF fp16 copies between DVE and
                    # ACT so the input stage drains faster
                    if c % 2 == 0:
                        nc.vector.tensor_copy(
                            xT_sb[0:CHUNK, c * W : (c + 1) * W], xt_ps[:, :]
                        )
                    else:
                        nc.scalar.copy(xT_sb[0:CHUNK, c * W : (c + 1) * W], xt_ps[:, :])
                    nc.vector.tensor_copy(
                        xTj_sb[0:CHUNK, c * JPC : (c + 1) * JPC],
                        xT_sb[0:CHUNK, c * W : c * W + JPC],
                    )
                    # S[k, i] = sum_d x[i,k,d] at partitions 32c+m
                    nc.tensor.matmul(
                        S_ps[32 * c : 32 * c + 32, :],
                        dmat_sb[:, 32:64],
                        xT_sb[0:CHUNK, c * W : (c + 1) * W],
                        start=True,
                        stop=True,
                        tile_position=(0, 32 * c),
                        skip_group_check=True,
                    )
                nc.scalar.copy(S16_sb[:, :], S_ps[:, :])
                # exp bias column: -S_j, upcast from the SAME fp16 S16 the
                # negI matmul reads so the diagonal cancels exactly
                nc.vector.tensor_scalar(
                    negSj_sb[:, :],
                    S16_sb[:, 0:JPC],
                    -1.0,
                    0.0,
                    mybir.AluOpType.mult,
                    mybir.AluOpType.bypass,
                )

            mainps_es = contextlib.ExitStack()
            mainps = mainps_es.enter_context(
                tc.tile_pool(name="mainps", bufs=1, space="PSUM")
            )
            NDIST = 6
            dist_bufs = [
                mainps.tile([128, W], mybir.dt.float32, name=f"dist{i}")
                for i in range(NDIST)
            ]

            # --- main loop over output rows ---
            # Row j only computes columns i >= j (ragged upper triangle):
            # the diagonal block's lower-triangle contributions come from the
            # colacc by symmetry (minus the double-counted self term 1.0,
            # subtracted on the host).
            for j in range(JPC):
                E = W if j < 32 else D4LO
                # colacc covers diag+k123 (0..256) for all rows, plus the
                # distance-4 high half (256..288) for rows j<32 only
                CP = D4LO if j < 32 else D4HI
                dist = dist_bufs[j % NDIST]
                for c in range(NCHUNK):
                    ab = ab_ring[j * NCHUNK + c]
                    # ab = relu(xT[:, i] - xT[:, j]) : (in - s1) max 0.0
                    # (const scalar2 keeps the second DVE read port free so
                    # the 4x perf mode applies)
                    nc.vector.tensor_scalar(
                        ab[:, :],
                        xT_sb[0:CHUNK, c * W + j : c * W + E],
                        xTj_sb[0:CHUNK, c * JPC + j : c * JPC + j + 1],
                        0.0,
                        mybir.AluOpType.subtract,
                        mybir.AluOpType.max,
                    )
                    # dist[32c+m, :] = 2 * sum_d ab[5m+d, :] (start=True:
                    # each quadrant's first write; -S added by negI below)
                    nc.tensor.matmul(
                        dist[32 * c : 32 * c + 32, j:E],
                        dmat_sb[:, 0:32],
                        ab[:, :],
                        start=True,
                        stop=False,
                        tile_position=(0, 32 * c),
                        skip_group_check=True,
                    )
                # dist += -S[k, i], LAST so early rows don't stall on the
                # S16 chain during the input stage
                nc.tensor.matmul(
                    dist[:, j:E],
                    negI_sb[:, :],
                    S16_sb[:, j:E],
                    start=False,
                    stop=True,
                    skip_group_check=True,
                )
                # dump = exp(-dist - S_j) = exp(-L1(i,j)) fp16;
                # accum_out gives the row sums over cols j..E
                nc.scalar.activation(
                    dump_ring[j % NDUMP][:, j:E],
                    dist[:, j:E],
                    mybir.ActivationFunctionType.Exp,
                    bias=negSj_sb[:, j : j + 1],
                    scale=-1.0,
                    accum_out=raw_sb[:, j : j + 1],
                )
                if j == 31:
                    # first half of the row sums is final: overlap its DMA
                    nc.sync.dma_start(out=rowsum[:, 0:32], in_=raw_sb[:, 0:32])
                # column sums on the otherwise-idle Pool (GPSIMD) engine:
                # colacc += dump[:, j:CP] (f32 accumulator in SBUF) — the
                # diag part doubles as the lower-triangle row sums and the
                # 256..288 part (rows <32 only) as the distance-4 exchange
                nc.gpsimd.tensor_tensor(
                    colacc_sb[:, j:CP],
                    colacc_sb[:, j:CP],
                    dump_ring[j % NDUMP][:, j:CP],
                    mybir.AluOpType.add,
                )

            mainps_es.close()
            nc.sync.dma_start(out=rowsum[:, 32:JPC], in_=raw_sb[:, 32:JPC])
            nc.gpsimd.dma_start(out=colout[:, :], in_=colacc_sb[:, :])

    nc.finalize()
    return nc


def _aux_consts():
    dm = np.zeros([CHUNK, 64], dtype=np.float16)
    for m in range(KPC):
        dm[5 * m : 5 * m + 5, m] = 2.0
        dm[5 * m : 5 * m + 5, 32 + m] = 1.0
    negI = (-np.eye(128)).astype(np.float16)
    return dm, negI


def make_in_maps(inputs, T):
    f16 = np.float16
    Tm = np.asarray(T, dtype=np.float32).astype(f16)
    dm, negI = _aux_consts()
    in_maps = []
    x = np.asarray(inputs, dtype=np.float32)
    for c in range(NCORES):
        rolled = np.roll(x, -JPC * c, axis=0)[0:W, :]
        # local col order: [diag+k123 (0..256) | c+4 rows 32..64 (256..288)
        # | c+4 rows 0..32 (288..320)]
        rolled = np.concatenate(
            [rolled[0:D4HI], rolled[D4LO:W], rolled[D4HI:D4LO]], axis=0
        )
        inTc = np.ascontiguousarray(rolled.T).astype(f16)
        in_maps.append(
            {
                "inT": inTc,
                "Tm": Tm,
                "dmat": dm,
                "negI": negI,
            }
        )
    return in_maps


def assemble_output(results):
    out = np.zeros([B, K], dtype=np.float32)
    # own row sums: raw[32c+m, j] -> out[64q+j, 25c+m]
    for q in range(NCORES):
        raw = np.asarray(results[q]["rowsum"], dtype=np.float32)  # [128, JPC]
        for cc in range(NCHUNK):
            out[JPC * q : JPC * (q + 1), KPC * cc : KPC * (cc + 1)] = raw[
                32 * cc : 32 * cc + KPC, :
            ].T
    # column sums: core b's group k serves rows of core b+k. k=0 is the own
    # diag block (lower triangle by symmetry; subtract the double-counted
    # self term exp(0)=1), k=1..3 are the exchanged off-diag groups.
    for b in range(NCORES):
        col = np.asarray(results[b]["colout"], dtype=np.float32)  # [128, NCOL]
        for k in range(0, NEX + 1):
            q = (b + k) % NCORES
            blk = col[:, JPC * k : JPC * (k + 1)]  # [128, JPC]
            for cc in range(NCHUNK):
                out[JPC * q : JPC * (q + 1), KPC * cc : KPC * (cc + 1)] += blk[
                    32 * cc : 32 * cc + KPC, :
                ].T
        # distance-4 high half: colsums over rows <32 of core b serve rows
        # 32..64 of core b+4
        q = (b + 4) % NCORES
        blk = col[:, D4HI:D4LO]  # [128, 32]
        for cc in range(NCHUNK):
            out[JPC * q + 32 : JPC * (q + 1), KPC * cc : KPC * (cc + 1)] += blk[
                32 * cc : 32 * cc + KPC, :
            ].T
    out -= 1.0
    return out


def kernel(inputs, T):
    from concourse.bass_utils import run_bass_kernel_spmd

    if "nc" not in _NC_CACHE:
        _NC_CACHE["nc"] = build_nc()
    nc = _NC_CACHE["nc"]
    in_maps = make_in_maps(inputs, T)
    res = run_bass_kernel_spmd(nc, in_maps, list(range(NCORES)))
    return assemble_output(res.results)


if __name__ == "__main__":
    sys.path.insert(0, "/root/problem")
    from reference import setup_inputs, reference

    inputs = setup_inputs()
    expected = np.asarray(reference(**inputs))
    actual = kernel(**{k: np.asarray(v) for k, v in inputs.items()})
    err = np.abs(actual - expected)
    rel = np.linalg.norm(actual - expected) / np.linalg.norm(expected)
    print(f"max abs err: {err.max():.3e}")
    print(f"Relative error: {rel:.3e}")


# revision 14
# speedup vs baseline: 1.1268x; 1.0681x over previous
"""
MinibatchDiscrimination kernel for 8x TRN2 NeuronCores (Bass/Tile).

Math:  x = inputs @ T  -> [B, K, D] with B=512, K=100, D=5
       out[i,k] = sum_j exp(-sum_d |x[i,k,d]-x[j,k,d]|)

Strategy — symmetric block-tournament over the pairwise matrix:

  The B x B pairwise matrix is tiled into 8x8 blocks of 64x64 (one row-group
  per core). Each unordered block-pair only needs computing once: from one
  computed block, ROW sums come from the ACT accumulator and COLUMN sums
  (= row sums of the transposed block, by symmetry of the L1 distance) come
  from a PE identity-matmul accumulation over the exp tiles. Core c computes
  blocks (c, c+k) for k=0..4 (mod 8, W=320 columns of its rolled copy):

    - diag block (k=0): row sums only (colsum would double-count by symmetry)
    - k=1,2,3: row sums kept locally + column sums exchanged to core c+k
      (exchange happens on the host during output assembly)
    - k=4: row sums only; the mirror pair {c, c+4} is computed independently
      by core c+4 as ITS k=4 block (distance-4 blocks are duplicated so the
      SPMD program stays identical across cores)

  Row j of core q then receives: own row sums (col-groups q..q+4) plus
  exchanged column sums from cores q-1, q-2, q-3 — all 8 groups exactly once.

Per core c of 8 (rolled by 64c so the program is SPMD-identical):
  - xT[kd, i] = sum_f T[f, kd] * inT[f, i] on PE (4 chunks of 125 kd), i<320.
  - Per output row j in 0..63:
      ab_c[p, i] = |xT_c[p, i] - xT_c[p, j]|   (DVE tensor_scalar
                   (subtract, abs_max vs 0.0) — fp16, 4x perf mode;
                   the per-partition scalar is an f32 upcast of the fp16 xT
                   column so the diagonal is exactly 0)
      dist[32c+m, :] = sum_d ab[5m+d, :]       (PE d-sum matmul with a
                   0/1 block matrix, col-tiled per chunk — no S-term or
                   negI matmul needed since abs values sum directly)
      dump[:, :]  = exp(-dist), fp16 -> SBUF   (ACT, accum_out gives the
                   row sums over all 320 cols in one pass)
      colacc     += dump[:, 64:256]            (PE identity matmul
                   accumulating in PSUM across all 64 j — the k=1,2,3
                   column sums, emitted 2 iterations late to pipeline)
  - dist row p=32c+m holds k=25c+m (m<25); host transposes/reassembles and
    adds the exchanged column-sum blocks.

  Hardware notes (CoreSim cost model, validated on TRN2 previously):
  - Steady state is ACT-bound: exp main pass 0.833*320+185 = 452ns plus the
    fixed 287ns accumulator-read = ~739ns/row; DVE 4x tensor_scalars at
    143.8ns = 575ns/row and PE 4 d-sums + colacc = 613ns/row overlap under
    it. 64 rows -> ~47us steady.
  - ab/dump tiles are STATIC rings sized to the whole loop (256 ab tiles,
    ~160KB of SBUF) so there are no cross-iteration WAW deps at all: DVE
    instructions carry no waits in steady state (the baseline lost ~10us+
    to 242 same-engine WAW EventSemaphores from rotating small rings).
  - Inputs land in 4 DMAs (two ~0.5-1MB strided transfers each for T/inT
    halves) so SP descriptor-gen time stays off the critical path; the ACT
    exp table is pre-warmed during the DMAs.
"""

import sys
import numpy as np

for _p in ("/opt/trn_rl_repo",):
    if _p not in sys.path:
        sys.path.insert(0, _p)

B = 512
F = 1024
K = 100
D = 5
KD = K * D  # 500
NCORES = 8
JPC = B // NCORES  # 64 output rows per core
NCHUNK = 4  # kd chunks of 125
CHUNK = KD // NCHUNK  # 125
KPC = K // NCHUNK  # 25 k's per chunk
NBLK = 5  # col block-groups computed per core (k = 0..4)
W = NBLK * JPC  # 320 pairwise columns per core
NEX = 3  # exchanged colsum groups (k = 1, 2, 3)
CEX = NEX * JPC  # 192 exchanged columns (local cols 64..256)
D4HI = 4 * JPC  # 256: start of the distance-4 high-half cols (rows 32..64 of c+4)
D4LO = D4HI + 32  # 288: start of the distance-4 low-half cols, DESCENDING:
#                   local col 288+s holds core c+4's row 31-s
NCOL = W  # 320 columns exported in colout
NRED = 20  # rows 64-NRED..63 get DVE tensor_reduce row sums (no ACT accum)

_NC_CACHE = {}


def build_nc():
    import contextlib

    import concourse.bass as bass
    import concourse.bacc as bacc
    import concourse.mybir as mybir
    from concourse.tile import TileContext

    nc = bacc.Bacc(None, target_bir_lowering=False, debug=True)

    inT = nc.declare_dram_parameter("inT", [F, W], mybir.dt.float16, isOutput=False)
    Tm = nc.declare_dram_parameter("Tm", [F, KD], mybir.dt.float16, isOutput=False)
    # dmat[5m+d, m] = 2.0 (d-sum of 2*relu), dmat[5m+d, 32+m] = 1.0 (S row sums)
    dmat = nc.declare_dram_parameter(
        "dmat", [CHUNK, 64], mybir.dt.float16, isOutput=False
    )
    negI = nc.declare_dram_parameter("negI", [128, 128], mybir.dt.float16, isOutput=False)
    rowsum = nc.declare_dram_parameter("rowsum", [128, JPC], mybir.dt.float32, isOutput=True)
    colout = nc.declare_dram_parameter(
        "colout", [128, NCOL], mybir.dt.float32, isOutput=True
    )

    with TileContext(nc) as tc:
        with tc.tile_pool(name="persist", bufs=1) as pp:
            T_sb = pp.tile([128, 8 * KD], mybir.dt.float16, name="T_sb")
            inT_sb = pp.tile([128, 8 * W], mybir.dt.float16, name="inT_sb")
            dmat_sb = pp.tile([CHUNK, 64], mybir.dt.float16, name="dmat_sb")
            S16_sb = pp.tile([128, W], mybir.dt.float16, name="S16_sb")
            negSj_sb = pp.tile([128, JPC], mybir.dt.float32, name="negSj_sb")
            colacc_sb = pp.tile([128, NCOL], mybir.dt.float32, name="colacc_sb")
            negI_sb = pp.tile([128, 128], mybir.dt.float16, name="negI_sb")
            xT_sb = pp.tile([128, NCHUNK * W], mybir.dt.float16, name="xT_sb")
            # f32 upcasts of xT columns 0..JPC (tensor_scalar per-partition
            # scalars must be f32). Upcast from the fp16 xT so the diagonal
            # |x - x| stays exactly zero.
            xTj_sb = pp.tile([128, NCHUNK * JPC], mybir.dt.float32, name="xTj_sb")
            raw_sb = pp.tile([128, JPC], mybir.dt.float32, name="raw_sb")

            warm_sb = pp.tile([1, 1], mybir.dt.float32, name="warm_sb")
            # Static rings: every (j, chunk) gets its own ab tile and every
            # j its own dump slot modulo 8 — cross-iteration WAW deps are
            # either absent (ab) or satisfied 8 iterations early (dump).
            # row j computes cols j..E(j). j>=32: E=288 (the low half of
            # the distance-4 block comes from core c-4's colacc). j<32:
            # E=320-j — the d4lo cols are DESCENDING (288+s holds c+4's row
            # 31-s), so the range covers partner rows t>=j; pairs with t<j
            # come from core c-4's colacc over its rows j'<i (strict, via
            # the colacc range ending at 319-j).
            def _erow(j):
                return (W - j) if j < 32 else D4LO

            ab_ring = [
                pp.tile(
                    [CHUNK, _erow(t // NCHUNK) - (t // NCHUNK)],
                    mybir.dt.float16,
                    name=f"ab{t}",
                )
                for t in range(JPC * NCHUNK)
            ]
            # ring must cover the tail rows whose sums are reduced on DVE
            # after the whole ts stream
            NDUMP = 24
            dump_ring = [
                pp.tile([128, W], mybir.dt.float16, name=f"dump{t}")
                for t in range(NDUMP)
            ]

            # --- load inputs: 2 halves each of T/inT so matmuls can start
            # after the first halves land, in 6 total strided DMAs ---
            # the cost of a DMA is charged to its issuing queue (SP/ACT/
            # Pool are the only DMA-capable queues): T halves run in
            # parallel on SP+ACT, inT halves on Pool, so the PE matmuls
            # start ~0.9us earlier
            def dma_T(e, h):
                e.dma_start(
                    out=T_sb[:, h * 4 * KD : (h + 1) * 4 * KD].rearrange(
                        "p (t k) -> p t k", t=4
                    ),
                    in_=Tm[h * 512 : (h + 1) * 512, :].rearrange(
                        "(t p) k -> p t k", t=4
                    ),
                )

            def dma_inT(e, h):
                e.dma_start(
                    out=inT_sb[:, h * 4 * W : (h + 1) * 4 * W].rearrange(
                        "p (t w) -> p t w", t=4
                    ),
                    in_=inT[h * 512 : (h + 1) * 512, :].rearrange(
                        "(t p) w -> p t w", t=4
                    ),
                )

            dma_T(nc.sync, 0)
            dma_inT(nc.gpsimd, 0)
            dma_T(nc.gpsimd, 1)
            dma_inT(nc.scalar, 1)
            nc.sync.dma_start(out=dmat_sb[:, :], in_=dmat[:, :])
            nc.sync.dma_start(out=negI_sb[:, :], in_=negI[:, :])
            # zero the Pool-side column-sum accumulator while DMAs run
            nc.gpsimd.memset(colacc_sb[:, :], 0.0)
            # warm the ACT exp table while the xT matmuls run (~1.3us)
            nc.vector.memset(warm_sb[:, :], 0.0)
            nc.scalar.activation(
                warm_sb[:, :], warm_sb[:, :], mybir.ActivationFunctionType.Exp
            )

            with tc.tile_pool(name="xtps", bufs=3, space="PSUM") as xtps:
                # "gate" matmuls: each absorbs exactly one input-DMA (or
                # const-DMA) semaphore inline, so no real matmul carries two
                # waits — a 2-wait matmul gets a separate EventSemaphore on
                # the PE queue which resets the p-state ramp clock, dropping
                # the whole input stage to half clock
                gate_srcs = [
                    T_sb[:, 0:64],
                    inT_sb[:, 0:64],
                    T_sb[:, 4 * KD : 4 * KD + 64],
                    inT_sb[:, 4 * W : 4 * W + 64],
                    dmat_sb[:, 0:64],
                    negI_sb[:, 0:64],
                ]
                g_ps0 = xtps.tile([128, 64], mybir.dt.float32, name="gate_a", bufs=1)
                g_ps1 = xtps.tile([128, 64], mybir.dt.float32, name="gate_b", bufs=1)
                for gi, gsrc in enumerate(gate_srcs):
                    g_ps = g_ps0 if gi < 4 else g_ps1
                    q = gi % 4
                    nc.tensor.matmul(
                        g_ps[32 * q : 32 * q + 32, :],
                        gsrc[:, 0:32],
                        gsrc[:, 0:64],
                        start=True,
                        stop=True,
                        tile_position=(0, 32 * q),
                        skip_group_check=True,
                    )
                # --- xT chunks: xT[kd, i] via PE over f tiles; the S-row-sum
                # quadrant matmul for each chunk is emitted right after its
                # copy so the S16/negSj chain never serializes at the end ---
                S_ps = xtps.tile([128, W], mybir.dt.float32, name="S_ps", bufs=1)
                for c in range(NCHUNK):
                    xt_ps = xtps.tile([CHUNK, W], mybir.dt.float32, name="xt_ps")
                    for t in range(8):
                        nc.tensor.matmul(
                            xt_ps[:, :],
                            T_sb[:, t * KD + c * CHUNK : t * KD + (c + 1) * CHUNK],
                            inT_sb[:, t * W : (t + 1) * W],
                            start=(t == 0),
                            stop=(t == 7),
                        )
                    # alternate the PSUM->SBUF fp16 copies between DVE and
                    # ACT so the input stage drains faster
                    if c % 2 == 0:
                        nc.vector.tensor_copy(
                            xT_sb[0:CHUNK, c * W : (c + 1) * W], xt_ps[:, :]
                        )
                    else:
                        nc.scalar.copy(xT_sb[0:CHUNK, c * W : (c + 1) * W], xt_ps[:, :])
                    nc.vector.tensor_copy(
                        xTj_sb[0:CHUNK, c * JPC : (c + 1) * JPC],
                        xT_sb[0:CHUNK, c * W : c * W + JPC],
                    )
                    # S[k, i] = sum_d x[i,k,d] at partitions 32c+m
                    nc.tensor.matmul(
                        S_ps[32 * c : 32 * c + 32, :],
                        dmat_sb[:, 32:64],
                        xT_sb[0:CHUNK, c * W : (c + 1) * W],
                        start=True,
                        stop=True,
                        tile_position=(0, 32 * c),
                        skip_group_check=True,
                    )
                nc.scalar.copy(S16_sb[:, :], S_ps[:, :])
                # exp bias column: -S_j, upcast from the SAME fp16 S16 the
                # negI matmul reads so the diagonal cancels exactly
                nc.vector.tensor_scalar(
                    negSj_sb[:, :],
                    S16_sb[:, 0:JPC],
                    -1.0,
                    0.0,
                    mybir.AluOpType.mult,
                    mybir.AluOpType.bypass,
                )

            mainps_es = contextlib.ExitStack()
            mainps = mainps_es.enter_context(
                tc.tile_pool(name="mainps", bufs=1, space="PSUM")
            )
            NDIST = 6
            dist_bufs = [
                mainps.tile([128, W], mybir.dt.float32, name=f"dist{i}")
                for i in range(NDIST)
            ]

            # --- main loop over output rows ---
            # Row j only computes columns i >= j (ragged upper triangle):
            # the diagonal block's lower-triangle contributions come from the
            # colacc by symmetry (minus the double-counted self term 1.0,
            # subtracted on the host).
            for j in range(JPC):
                E = _erow(j)
                # colacc covers diag+k123 (0..256) for all rows; rows j<32
                # also accumulate the distance-4 high half (256..288) and
                # the d4lo cols STRICTLY above the antidiagonal (..319-j)
                CP = (W - 1 - j) if j < 32 else D4HI
                dist = dist_bufs[j % NDIST]
                for c in range(NCHUNK):
                    ab = ab_ring[j * NCHUNK + c]
                    # ab = relu(xT[:, i] - xT[:, j]) : (in - s1) max 0.0
                    # (const scalar2 keeps the second DVE read port free so
                    # the 4x perf mode applies)
                    nc.vector.tensor_scalar(
                        ab[:, :],
                        xT_sb[0:CHUNK, c * W + j : c * W + E],
                        xTj_sb[0:CHUNK, c * JPC + j : c * JPC + j + 1],
                        0.0,
                        mybir.AluOpType.subtract,
                        mybir.AluOpType.max,
                    )
                    # dist[32c+m, :] = 2 * sum_d ab[5m+d, :] (start=True:
                    # each quadrant's first write; -S added by negI below)
                    nc.tensor.matmul(
                        dist[32 * c : 32 * c + 32, j:E],
                        dmat_sb[:, 0:32],
                        ab[:, :],
                        start=True,
                        stop=False,
                        tile_position=(0, 32 * c),
                        skip_group_check=True,
                    )
                # dist += -S[k, i], LAST so early rows don't stall on the
                # S16 chain during the input stage
                nc.tensor.matmul(
                    dist[:, j:E],
                    negI_sb[:, :],
                    S16_sb[:, j:E],
                    start=False,
                    stop=True,
                    skip_group_check=True,
                )
                # dump = exp(-dist - S_j) = exp(-L1(i,j)) fp16; ACT accum
                # gives the row sums except for the last NRED rows, whose
                # sums are DVE tensor_reduces of the fp16 dump (saves the
                # fixed 187ns ACT accumulator-read on those rows)
                nc.scalar.activation(
                    dump_ring[j % NDUMP][:, j:E],
                    dist[:, j:E],
                    mybir.ActivationFunctionType.Exp,
                    bias=negSj_sb[:, j : j + 1],
                    scale=-1.0,
                    accum_out=(raw_sb[:, j : j + 1] if j < JPC - NRED else None),
                )
                if j == 31:
                    # first half of the row sums is final: overlap its DMA
                    nc.sync.dma_start(out=rowsum[:, 0:32], in_=raw_sb[:, 0:32])
                # column sums on the otherwise-idle Pool (GPSIMD) engine:
                # colacc += dump[:, j:CP] (f32 accumulator in SBUF) — the
                # diag part doubles as the lower-triangle row sums and the
                # 256..288 part (rows <32 only) as the distance-4 exchange
                nc.gpsimd.tensor_tensor(
                    colacc_sb[:, j:CP],
                    colacc_sb[:, j:CP],
                    dump_ring[j % NDUMP][:, j:CP],
                    mybir.AluOpType.add,
                )

            for j in range(JPC - NRED, JPC):
                nc.vector.tensor_reduce(
                    raw_sb[:, j : j + 1],
                    dump_ring[j % NDUMP][:, j : _erow(j)],
                    mybir.AxisListType.X,
                    mybir.AluOpType.add,
                )
            mainps_es.close()
            nc.sync.dma_start(out=rowsum[:, 32:JPC], in_=raw_sb[:, 32:JPC])
            nc.gpsimd.dma_start(out=colout[:, :], in_=colacc_sb[:, :])

    nc.finalize()
    return nc


def _aux_consts():
    dm = np.zeros([CHUNK, 64], dtype=np.float16)
    for m in range(KPC):
        dm[5 * m : 5 * m + 5, m] = 2.0
        dm[5 * m : 5 * m + 5, 32 + m] = 1.0
    negI = (-np.eye(128)).astype(np.float16)
    return dm, negI


def make_in_maps(inputs, T):
    f16 = np.float16
    Tm = np.asarray(T, dtype=np.float32).astype(f16)
    dm, negI = _aux_consts()
    in_maps = []
    x = np.asarray(inputs, dtype=np.float32)
    for c in range(NCORES):
        rolled = np.roll(x, -JPC * c, axis=0)[0:W, :]
        # local col order: [diag+k123 (0..256) | c+4 rows 32..64 (256..288)
        # | c+4 rows 31..0 descending (288..320)]
        rolled = np.concatenate(
            [rolled[0:D4HI], rolled[D4LO:W], rolled[D4HI:D4LO][::-1]], axis=0
        )
        inTc = np.ascontiguousarray(rolled.T).astype(f16)
        in_maps.append(
            {
                "inT": inTc,
                "Tm": Tm,
                "dmat": dm,
                "negI": negI,
            }
        )
    return in_maps


def assemble_output(results):
    out = np.zeros([B, K], dtype=np.float32)
    # own row sums: raw[32c+m, j] -> out[64q+j, 25c+m]
    for q in range(NCORES):
        raw = np.asarray(results[q]["rowsum"], dtype=np.float32)  # [128, JPC]
        for cc in range(NCHUNK):
            out[JPC * q : JPC * (q + 1), KPC * cc : KPC * (cc + 1)] = raw[
                32 * cc : 32 * cc + KPC, :
            ].T
    # column sums: core b's group k serves rows of core b+k. k=0 is the own
    # diag block (lower triangle by symmetry; subtract the double-counted
    # self term exp(0)=1), k=1..3 are the exchanged off-diag groups.
    for b in range(NCORES):
        col = np.asarray(results[b]["colout"], dtype=np.float32)  # [128, NCOL]
        for k in range(0, NEX + 1):
            q = (b + k) % NCORES
            blk = col[:, JPC * k : JPC * (k + 1)]  # [128, JPC]
            for cc in range(NCHUNK):
                out[JPC * q : JPC * (q + 1), KPC * cc : KPC * (cc + 1)] += blk[
                    32 * cc : 32 * cc + KPC, :
                ].T
        # distance-4 high half: colsums over rows <32 of core b serve rows
        # 32..64 of core b+4; low half (descending cols): col 288+s serves
        # row 31-s with colsums over rows j<31-s (strict antidiagonal)
        q = (b + 4) % NCORES
        blk = col[:, D4HI:D4LO]  # [128, 32]
        for cc in range(NCHUNK):
            out[JPC * q + 32 : JPC * (q + 1), KPC * cc : KPC * (cc + 1)] += blk[
                32 * cc : 32 * cc + KPC, :
            ].T
        blk = col[:, D4LO:W][:, ::-1]  # reversed: col t serves row t of b+4
        for cc in range(NCHUNK):
            out[JPC * q : JPC * q + 32, KPC * cc : KPC * (cc + 1)] += blk[
                32 * cc : 32 * cc + KPC, :
            ].T
    out -= 1.0
    return out


def kernel(inputs, T):
    from concourse.bass_utils import run_bass_kernel_spmd

    if "nc" not in _NC_CACHE:
        _NC_CACHE["nc"] = build_nc()
    nc = _NC_CACHE["nc"]
    in_maps = make_in_maps(inputs, T)
    res = run_bass_kernel_spmd(nc, in_maps, list(range(NCORES)))
    return assemble_output(res.results)


if __name__ == "__main__":
    sys.path.insert(0, "/root/problem")
    from reference import setup_inputs, reference

    inputs = setup_inputs()
    expected = np.asarray(reference(**inputs))
    actual = kernel(**{k: np.asarray(v) for k, v in inputs.items()})
    err = np.abs(actual - expected)
    rel = np.linalg.norm(actual - expected) / np.linalg.norm(expected)
    print(f"max abs err: {err.max():.3e}")
    print(f"Relative error: {rel:.3e}")
